# revision 2
# baseline (speedup 1.0000x reference)
"""AutoRegressiveLSTM Trainium2 kernel v2 (8-core data-parallel, 4x PE
column tiling).

Per core: batch shard of 16 (padded to 32 = one PE column-tile).

Structure per step (vs v1's single 128x32-used PE):
  - The PE runs in 128x32 column-tiled mode: 4 concurrent tiles, one per
    PSUM partition quadrant. Quadrant qd computes, for all 4 gate types,
    the gate columns G with (G mod 128) in [32qd, 32qd+32).
  - Gate PSUM layout (per step, parity-alternating bank pairs):
    [128 part = (qd, batch), 1024 free = type(f,i,o,g) x (kk, a)] where
    gate index within type q = 128*kk + 32*qd + a.
  - Cell math runs on all 128 partitions (4x fewer cycles than v1):
    ACT: sigmoid in place on [128, 768] (f,i,o), tanh g -> SBUF.
    DVE: c = sig_f*c; tmp = sig_i*tanh_g; c += tmp; ACT tanh c;
    DVE h(f16) = sig_o * tanh_c.
  - ONE nc.vector.transpose (32x32 block transpose) of h [128,256] yields
    hT [128, 8 k-tiles x 32 batch] directly = next step's stationaries.
    No PE transposes, no tiling-mode switches.
  - Bias is added via a K=128 matmul round with a one-hot row stationary.
  - x-round and bias-round of step t carry start=True (first writes per
    bank) and execute during step t-1's cell math (PE tail filling).
  - Output feedback folded into W_eff as in v1; t=0 adds an o0 round.
  - Final phase: y^T = W_out @ H^T batched over steps (unchanged from v1).
"""

import numpy as np

import concourse.bass as bass
import concourse.mybir as mybir
import concourse.tile as tile
from concourse import bacc
from concourse.bass_utils import run_bass_kernel_spmd

F32 = mybir.dt.float32
F16 = mybir.dt.float16
AF = mybir.ActivationFunctionType
ET = mybir.EngineType

B, T, I, H, O = 128, 1024, 128, 1024, 128
NCORES = 8
BS = B // NCORES          # real batch per core (16)
BP = 32                   # padded batch per core (one column tile)
NK = H // 128             # 8 h k-tiles
TYPE_PERM = [1, 0, 2, 3]  # our order [f,i,g,o] in reference order i,f,g,o
# bank0 = {f,i}: cell part 1 (sigmoid + c*sig_f) overlaps bank1's matmuls


def build_program(t_steps=T + 1, unroll=8, final_tblk=32, per_quad_start=True,
                  debug_taps=False, ablate=()):
    """t_steps includes t=0; the loop runs (t_steps-1) steps (divisible by
    unroll). per_quad_start: A/B flag for PSUM has_written semantics."""
    assert (t_steps - 1) % unroll == 0
    n_iters = (t_steps - 1) // unroll
    if t_steps - 1 > 0:
        final_tblk = min(final_tblk, t_steps - 1)
        assert (t_steps - 1) % final_tblk == 0

    nc = bacc.Bacc(
        "TRN2", target_bir_lowering=False, debug=False, num_devices=NCORES
    )

    xT = nc.dram_tensor("xT", [128, (t_steps + unroll) * BP], F16, kind="ExternalInput")
    Wstr = nc.dram_tensor("Wstr", [128, NK * 4 * 1024], F16, kind="ExternalInput")
    Wx = nc.dram_tensor("Wx", [128, 4 * 1024], F16, kind="ExternalInput")
    Wio = nc.dram_tensor("Wio", [128, 4 * 1024], F16, kind="ExternalInput")
    WoT = nc.dram_tensor("WoT", [128, NK * O], F16, kind="ExternalInput")
    biasm = nc.dram_tensor("biasm", [128, 4 * 1024], F16, kind="ExternalInput")
    onehot = nc.dram_tensor("onehot", [128, BP], F16, kind="ExternalInput")
    hT0 = nc.dram_tensor("hT0", [128, NK * BP], F16, kind="ExternalInput")
    o0T = nc.dram_tensor("o0T", [128, BP], F16, kind="ExternalInput")
    c0pk = nc.dram_tensor("c0pk", [128, 256], F16, kind="ExternalInput")

    HT = nc.dram_tensor("HT", [t_steps, 128, NK * BP], F16)  # internal staging
    yT = nc.dram_tensor("yT", [128, t_steps * BS], F32, kind="ExternalOutput")
    if debug_taps:
        dbg_g0 = nc.dram_tensor("dbg_g0", [128, 1024], F32, kind="ExternalOutput")
        dbg_h0 = nc.dram_tensor("dbg_h0", [128, 256], F32, kind="ExternalOutput")
        dbg_hT0 = nc.dram_tensor("dbg_hT0", [128, 256], F32, kind="ExternalOutput")

    with tile.TileContext(nc) as tc:
        from contextlib import ExitStack

        with ExitStack() as perm:
            pw = perm.enter_context(tc.tile_pool(name="weights", bufs=1))
            Wstr_sb = pw.tile([128, NK * 4 * 1024], F16)
            WoT_sb = pw.tile([128, NK * O], F16)
            nc.sync.dma_start(Wstr_sb[:], Wstr[:])
            nc.sync.dma_start(WoT_sb[:], WoT[:])

            with ExitStack() as rec:
                pr = rec.enter_context(tc.tile_pool(name="recur", bufs=1))
                Wx_sb = pr.tile([128, 4 * 1024], F16)
                Wio_sb = pr.tile([128, 4 * 1024], F16)
                biasm_sb = pr.tile([128, 4 * 1024], F16)
                oh_sb = pr.tile([128, BP], F16)
                x_sb = pr.tile([128, unroll * BP], F16)
                x0_sb = pr.tile([128, BP], F16)
                o0_sb = pr.tile([128, BP], F16)
                hT_sb = [pr.tile([128, NK * BP], F16, name=f"hT{p}")
                         for p in range(2)]
                c_sb = pr.tile([128, 256], F16)
                tg_sb = pr.tile([128, 256], F16)
                tc_sb = pr.tile([128, 256], F16)
                tmp_sb = pr.tile([128, 256], F16)
                h_sb = pr.tile([128, 256], F16)

                pp = rec.enter_context(tc.tile_pool(name="gpsum", bufs=1, space="PSUM"))
                ps = pp.tile([128, 2048], F32, name="gps")  # banks 0-3

                nc.sync.dma_start(Wx_sb[:], Wx[:])
                nc.sync.dma_start(Wio_sb[:], Wio[:])
                nc.sync.dma_start(biasm_sb[:], biasm[:])
                nc.sync.dma_start(oh_sb[:], onehot[:])
                nc.sync.dma_start(x0_sb[:], xT[:, 0:BP])
                nc.sync.dma_start(o0_sb[:], o0T[:])
                nc.sync.dma_start(hT_sb[1][:], hT0[:])
                nc.sync.dma_start(c_sb[:], c0pk[:])
                if n_iters > 0:
                    # prefetch body 0's x (steps 1..unroll)
                    nc.sync.dma_start(x_sb[:], xT[:, BP:(unroll + 1) * BP])

                def mm(po, qd, half, stat, rhs, start, stop):
                    nc.tensor.matmul(
                        ps[32 * qd:32 * qd + 32, po + 512 * half:po + 512 * half + 512],
                        stat, rhs,
                        start=start, stop=stop,
                        tile_position=(0, 32 * qd),
                        skip_group_check=True,
                    )

                def head_rounds(t_par, x_stat, extra_o0=False):
                    """bias-round + x-round (+o0 at t=0): first writes of the
                    step's banks; independent of previous step's cell math."""
                    po = 1024 * t_par
                    for qd in range(4):
                        st = (qd == 0) or per_quad_start
                        for half in range(2):
                            rhs = biasm_sb[:, (2 * qd + half) * 512:(2 * qd + half + 1) * 512]
                            mm(po, qd, half, oh_sb[:], rhs, st, False)
                    for qd in range(4):
                        for half in range(2):
                            rhs = Wx_sb[:, (2 * qd + half) * 512:(2 * qd + half + 1) * 512]
                            mm(po, qd, half, x_stat, rhs, False, False)
                    if extra_o0:
                        for qd in range(4):
                            for half in range(2):
                                rhs = Wio_sb[:, (2 * qd + half) * 512:(2 * qd + half + 1) * 512]
                                mm(po, qd, half, o0_sb[:], rhs, False, False)

                def h_rounds(t_par, hT_prev):
                    """Bank0 pass (f,i gates) fully first, then bank1 (g,o):
                    bank0's cell math overlaps bank1's matmuls."""
                    if "hrounds" in ablate:
                        return
                    if "cell" in ablate:
                        hT_prev = hT_sb[1]  # constant stationary
                    po = 1024 * t_par
                    for half in range(2):
                        for kk in range(NK):
                            stat = hT_prev[:, 32 * kk:32 * kk + 32]
                            last = kk == NK - 1
                            for qd in range(4):
                                rhs = Wstr_sb[
                                    :, ((kk * 4 + qd) * 2 + half) * 512:
                                       ((kk * 4 + qd) * 2 + half + 1) * 512]
                                mm(po, qd, half, stat, rhs, False,
                                   last and qd == 3)

                def cell(t_par, t_out, t_ap):
                    """t_out: parity buffer to write hT into; t_ap: HT index.
                    Layout: f [0:256], i [256:512] (bank0); g [512:768],
                    o [768:1024] (bank1)."""
                    if "cell" in ablate:
                        return
                    po = 1024 * t_par
                    # part 1 (under bank1 MMs): sigmoid f,i; c = sig_f * c
                    nc.scalar.activation(ps[:, po:po + 512], ps[:, po:po + 512],
                                         AF.Sigmoid)
                    nc.vector.tensor_mul(c_sb[:], c_sb[:], ps[:, po:po + 256])
                    # part 2 (after bank1): tanh g, sigmoid o
                    nc.scalar.activation(tg_sb[:], ps[:, po + 512:po + 768],
                                         AF.Tanh)
                    nc.scalar.activation(ps[:, po + 768:po + 1024],
                                         ps[:, po + 768:po + 1024], AF.Sigmoid)
                    nc.vector.tensor_mul(tmp_sb[:], ps[:, po + 256:po + 512],
                                         tg_sb[:])
                    nc.vector.tensor_add(c_sb[:], c_sb[:], tmp_sb[:])
                    nc.scalar.activation(tc_sb[:], c_sb[:], AF.Tanh)
                    # h = sig_o * tanh(c)  (f16)
                    nc.vector.tensor_mul(h_sb[:], ps[:, po + 768:po + 1024],
                                         tc_sb[:])
                    # hT via DVE 32x32 block transpose
                    nc.vector.transpose(hT_sb[t_out][:], h_sb[:])
                    nc.sync.dma_start(HT[t_ap], hT_sb[t_out][:])

                # ---- t = 0 (parity 0) ----
                head_rounds(0, x0_sb[:], extra_o0=True)
                h_rounds(0, hT_sb[1])
                if debug_taps:
                    g0_sb = pr.tile([128, 1024], F32, name="g0dbg")
                    nc.vector.tensor_copy(g0_sb[:], ps[:, 0:1024])
                    nc.sync.dma_start(dbg_g0[:], g0_sb[:])
                # (par, out_buf) of the step whose cell math is still pending
                cell_pending = (0, 0)

                if n_iters > 0:
                    with tc.For_i(0, n_iters, 1, hint_engines=(ET.PE,)) as iv:
                        for s in range(unroll):
                            t_par = (1 + s) % 2
                            head_rounds(t_par, x_sb[:, s * BP:(s + 1) * BP])
                            # cell math of step (t-1) runs under these MMs
                            pp_par, pp_out = cell_pending
                            cell(pp_par, pp_out, iv * unroll + s)
                            h_rounds(t_par, hT_sb[pp_out])
                            cell_pending = (t_par, 1 - pp_out)
                        # prefetch next body's x
                        nc.sync.dma_start(
                            x_sb[:],
                            xT[:, bass.ds(((iv + 1) * unroll + 1) * BP,
                                          unroll * BP)]
                        )

                # final step's cell math (t = t_steps-1)
                fp_par, fp_out = cell_pending
                cell(fp_par, fp_out, t_steps - 1)
                if debug_taps:
                    h0f_sb = pr.tile([128, 256], F32, name="h0dbg")
                    hT0f_sb = pr.tile([128, 256], F32, name="hT0dbg")
                    nc.vector.tensor_copy(h0f_sb[:], h_sb[:])
                    nc.sync.dma_start(dbg_h0[:], h0f_sb[:])
                    nc.vector.tensor_copy(hT0f_sb[:], hT_sb[fp_out][:])
                    nc.sync.dma_start(dbg_hT0[:], hT0f_sb[:])

            # ---------------- final phase: y^T = W_out @ H^T ----------------
            with ExitStack() as fin:
                pf = fin.enter_context(tc.tile_pool(name="final", bufs=2))
                pfp = fin.enter_context(tc.tile_pool(name="ypsum", bufs=2, space="PSUM"))
                HT_r = HT[:].rearrange("t p (k b) -> p t k b", b=BP)
                for blk in range((t_steps - 1) // final_tblk):
                    t0 = blk * final_tblk
                    hblk = pf.tile([128, final_tblk, NK, BP], F16, name="hblk")
                    nc.sync.dma_start(hblk[:], HT_r[:, t0:t0 + final_tblk])
                    yps = pfp.tile([128, final_tblk * BS], F32, name="yps")
                    for k in range(NK):
                        nc.tensor.matmul(
                            yps[:],
                            WoT_sb[:, k * O:(k + 1) * O],
                            hblk[:, :, k, 0:BS],
                            start=(k == 0),
                            stop=(k == NK - 1),
                        )
                    y_sb = pf.tile([128, final_tblk * BS], F32, name="ysb")
                    nc.vector.tensor_copy(y_sb[:], yps[:])
                    nc.sync.dma_start(yT[:, t0 * BS:(t0 + final_tblk) * BS], y_sb[:])

    return nc


# ----------------------------------------------------------------------------
# Host-side packing
# ----------------------------------------------------------------------------

def pack_shared(W_ih, W_hh, b_ih, b_hh, W_out, b_out):
    f32, f16 = np.float32, np.float16
    W_ihx = W_ih[:, :I]
    W_io = W_ih[:, I:I + O]
    W_eff = (W_hh.astype(np.float64)
             + W_io.astype(np.float64) @ W_out.astype(np.float64)).astype(f32)
    b_eff = (b_ih.astype(np.float64) + b_hh.astype(np.float64)
             + W_io.astype(np.float64) @ b_out.astype(np.float64)).astype(f32)

    def pack_moving(Wt):
        # Wt [K, 4096] -> per (k-tile if K>128 else single, qd) [128, 1024]
        K = Wt.shape[0]
        nkk = K // 128
        A = Wt.reshape(nkk, 128, 4, NK, 4, 32)      # kkk ksub TT kk qd a
        A = A[:, :, TYPE_PERM][..., :, :, :]         # type order [f,i,o,g]
        # -> [kkk, qd, ksub, T, kk, a]
        Ap = A.transpose(0, 4, 1, 2, 3, 5)
        return np.ascontiguousarray(
            Ap.reshape(nkk, 4, 128, 1024)).astype(f16)

    Wstr_p = pack_moving(W_eff.T)                    # [8, 4, 128, 1024]
    # layout in SBUF free dim: ((kkk*4+qd)*2+half)*512 + c
    Wstr_out = np.empty((128, NK * 4 * 1024), f16)
    for kkk in range(NK):
        for qd in range(4):
            Wstr_out[:, (kkk * 4 + qd) * 1024:(kkk * 4 + qd + 1) * 1024] = \
                Wstr_p[kkk, qd]

    def pack_single(Wt):
        P = pack_moving(Wt)                          # [1, 4, 128, 1024]
        out = np.empty((128, 4 * 1024), f16)
        for qd in range(4):
            out[:, qd * 1024:(qd + 1) * 1024] = P[0, qd]
        return out

    D = b_eff.reshape(4, NK, 4, 32)[TYPE_PERM]       # T kk qd a
    biasm = np.zeros((128, 4 * 1024), f16)
    for qd in range(4):
        biasm[0, qd * 1024:(qd + 1) * 1024] = \
            D[:, :, qd, :].reshape(1024).astype(f16)
    onehot = np.zeros((128, BP), f16)
    onehot[0, :] = 1.0

    WoT_p = np.empty((128, NK * O), f16)
    for k in range(NK):
        WoT_p[:, k * O:(k + 1) * O] = W_out[:, 128 * k:128 * (k + 1)].T.astype(f16)

    return {
        "Wstr": Wstr_out, "Wx": pack_single(W_ihx.T),
        "Wio": pack_single(W_io.T), "WoT": WoT_p,
        "biasm": biasm, "onehot": onehot,
    }


def make_core_inputs(xc, hnc, cnc, o0c_raw, W_out, b_out, t_steps, shared,
                     unroll=8):
    f16 = np.float16
    T_in = xc.shape[1]
    xTc = np.zeros((128, (t_steps + unroll) * BP), f16)
    xTc.reshape(128, t_steps + unroll, BP)[:, :T_in, :BS] = \
        xc.transpose(2, 1, 0).astype(f16)
    hTc = np.zeros((128, NK * BP), f16)
    hTc.reshape(128, NK, BP)[:, :, :BS] = \
        hnc.T.reshape(NK, 128, BS).transpose(1, 0, 2).astype(f16)
    o0_corr = o0c_raw - (hnc @ W_out.T + b_out)
    o0c = np.zeros((128, BP), f16)
    o0c[:, :BS] = o0_corr.T.astype(f16)
    # c0 [128, 256]: part (i, b), free (kk, a); c[b, 128kk+32i+a]
    c0c = np.zeros((128, 256), f16)
    c0c.reshape(4, 32, NK, 32)[:, :BS] = \
        cnc.reshape(BS, NK, 4, 32).transpose(2, 0, 1, 3).astype(f16)
    out = {"xT": xTc, "hT0": hTc, "o0T": o0c, "c0pk": c0c}
    out.update(shared)
    return out


def _pack_inputs(x, sequence_length, hn, cn, output_t,
                 W_ih, W_hh, b_ih, b_hh, W_out, b_out, t_steps, unroll=8):
    f32 = np.float32
    x = np.asarray(x, f32)
    hn = np.asarray(hn, f32)
    cn = np.asarray(cn, f32)
    output_t = np.asarray(output_t, f32)
    W_ih = np.asarray(W_ih, f32)
    W_hh = np.asarray(W_hh, f32)
    b_ih = np.asarray(b_ih, f32)
    b_hh = np.asarray(b_hh, f32)
    W_out = np.asarray(W_out, f32)
    b_out = np.asarray(b_out, f32)
    seq = np.asarray(sequence_length).astype(np.int64)

    Tn = x.shape[1]
    mask = (np.arange(Tn)[None, :] < seq[:, None])
    xm = x * mask[:, :, None].astype(f32)

    shared = pack_shared(W_ih, W_hh, b_ih, b_hh, W_out, b_out)
    per_core = []
    for core in range(NCORES):
        bsl = slice(core * BS, (core + 1) * BS)
        per_core.append(make_core_inputs(
            xm[bsl], hn[bsl], cn[bsl], output_t[bsl],
            W_out, b_out, t_steps, shared, unroll=unroll))
    return per_core, mask, b_out


_CACHE = {}


def kernel(**inputs) -> np.ndarray:
    t_steps = T + 1
    key = ("nc", t_steps)
    if key not in _CACHE:
        nc_new = build_program(t_steps=t_steps)
        nc_new.compile()
        _CACHE[key] = nc_new
    nc = _CACHE[key]

    per_core, mask, b_out = _pack_inputs(t_steps=t_steps, **inputs)
    res = run_bass_kernel_spmd(nc, per_core, core_ids=list(range(NCORES)))

    Tn = mask.shape[1]
    y = np.empty((B, Tn, O), dtype=np.float32)
    for core in range(NCORES):
        yTc = res.results[core]["yT"]
        yc = yTc.reshape(128, t_steps, BS)[:, :Tn, :]
        y[core * BS:(core + 1) * BS] = yc.transpose(2, 1, 0)
    y += np.asarray(b_out, np.float32)[None, None, :]
    y *= mask[:, :, None].astype(np.float32)
    return y
